# revision 18
# baseline (speedup 1.0000x reference)
"""ACT-LSTM (adaptive computation time) Bass/Tile kernel for 8 TRN2 NeuronCores.

Model (per batch row, up to 8 iterations):
    4 LSTM gates:  g = act(x @ Wx_g.T + bx_g + state @ Wh_g.T + bh_g)
    cell  = f*cell + i*c ; state = o*tanh(cell)
    out   = sigmoid(relu(state @ W1.T + b1) @ W2.T + b2)
    h     = sigmoid(state @ W_halt.T + b_halt); rows halt when cumsum(h) >= 1-eps
    final = sum_t out_t * halt_weight_t

Strategy:
  - Data-parallel: batch 8192 sharded to 8 cores (1024 rows each). Weights replicated.
  - Transposed layout everywhere: hidden dim on SBUF partitions, batch on the free
    dim, so the recurrent matmul needs no per-iteration transposes.
  - fp8(e4m3) DoubleRowSwInterleave matmuls for every projection: the 4
    recurrent gate chains, the x-projection (folded into the same chain via a
    duplicated-x trick: x sits twice on the 128 partitions with half-weights),
    the W1 MLP projection, and the halt projection (chunked DoubleRow - a
    2-col SwInterleave LDWEIGHTS fails walrus). Weights are pre-scaled and
    pre-interleaved on host (pairs interleaved, columns reversed) so LDWEIGHTS
    reads contiguously; state stored fp8 x32 in [128, 8, BL] k-major layout;
    the whole PSUM chain carries one 2^15 scale, removed by the activation's
    scale=2^-15. Halting decisions are self-correcting (flipped rows carry
    ~1-p_sum ~ 1e-3 of mass), so fp8 noise is benign: rel err ~6.5e-4.
  - Weight-paired dense iterations: both 512-col batch blocks are streamed per
    loaded weight tile, amortizing DoubleRow's (FWL-less) weight loads.
  - t=0 specialization: state==0, so gates need only the x-projection.
  - Ragged-sequence exploitation: after t=1 the still-active rows are compacted
    (stream compaction via GPSIMD local_scatter with prefix-sum indices), so
    iteration 2 runs on <=512 columns instead of 1024. Iterations 3..7 are guarded
    by data-dependent Ifs and are skipped entirely once every row has halted.
  - Per-batch-row scalars (p_sum/act/acc/...) are packed on separate [1, BL] rows.
"""

import sys

sys.path.insert(0, "/opt/trn_rl_repo")

import numpy as np
import ml_dtypes

BATCH, IN, HID, OMID = 8192, 64, 1024, 128
MAX_ITER = 8
EPS = 1e-3
NCORES = 8
BL = BATCH // NCORES          # 1024 rows per core
NBLK = 512                    # matmul moving-dim (batch) block
NBLKS = BL // NBLK            # 2
PT = 128
KT = HID // PT                # 8 contraction tiles
JT = HID // PT                # 8 output-row tiles
KP = KT // 2                  # 4 DoubleRow contraction pairs

WS = 1024.0                   # fp8 weight scale
SS = 32.0                     # fp8 state scale
SCL = 1.0 / (WS * SS)         # 2^-15: descale applied in activations
XSC = 48.0                    # fp8 x scale
WXS = (WS * SS) / XSC / 2.0   # fp8 x-weight scale (dup trick halves weights)

_CACHE: dict = {}


def _build_nc(reps=1, upto='full', with_lib=True, opts=None):
    opts = dict(dict(gates_first=True, ft_hints=True, pair=True, swil=True,
                     no_mlp=False),
                **(opts or {}))
    import concourse.mybir as mybir
    from concourse import bacc, library_config
    from concourse.tile import TileContext

    f32 = mybir.dt.float32
    bf16 = mybir.dt.bfloat16
    fp8 = mybir.dt.float8e4
    i16 = mybir.dt.int16
    i32 = mybir.dt.int32
    AF = mybir.ActivationFunctionType
    ALU = mybir.AluOpType
    DR = mybir.MatmulPerfMode.DoubleRow
    DRS = mybir.MatmulPerfMode.DoubleRowSwInterleave
    SWIL = opts["swil"]

    nc = bacc.Bacc("TRN2", target_bir_lowering=False, debug=False,
                   enable_asserts=False)

    GATES = "ifco"
    # ---- DRAM parameters (all pre-transposed / pre-cast on host) ----
    d_xT = nc.dram_tensor("xT", [2 * IN, BL], bf16, kind="ExternalInput")
    # fp8 x in DoubleRow moving layout: group0 = x.T duplicated (x48), group1 = 0
    d_x8 = nc.dram_tensor("x8", [PT, 2, BL], fp8, kind="ExternalInput")
    # fp8 SwInterleave x-weights (dup trick: [p, jt, 2j] = Wx.T[p%64, jt*128+127-j]*WXS)
    d_Wx8 = {g: nc.dram_tensor(f"Wx8s{g}", [PT, JT, 2 * PT], fp8,
                               kind="ExternalInput") for g in GATES}
    if SWIL:
        # software-interleaved DoubleRow weights: [p, kp*JT+jt, 2j] =
        # Wh.T[2kp*128+p, jt*128+127-j]*WS, [.., 2j+1] = Wh.T[(2kp+1)*128+p, ..]
        d_Wh8 = {g: nc.dram_tensor(f"Wh8s{g}", [PT, KP * JT, 2 * PT], fp8,
                                   kind="ExternalInput") for g in GATES}
        d_W18 = nc.dram_tensor("W18s", [PT, KP, 2 * OMID], fp8,
                               kind="ExternalInput")
        # halt stays chunked-DR: a 2-active-col SwInterleave LDWEIGHTS fails
        # walrus's s3_lw_valid_num_active_cols check
        d_Whalt8 = nc.dram_tensor("Whalt8", [PT, KT, 16], fp8,
                                  kind="ExternalInput")
    else:
        # chunked DoubleRow weights: [p, k, j] = Wh.T[k*128+p, j] * WS
        d_Wh8 = {g: nc.dram_tensor(f"Wh8{g}", [PT, KT, HID], fp8,
                                   kind="ExternalInput") for g in GATES}
        d_W18 = nc.dram_tensor("W18", [PT, KT, OMID], fp8, kind="ExternalInput")
        d_Whalt8 = nc.dram_tensor("Whalt8", [PT, KT, 16], fp8,
                                  kind="ExternalInput")
    # WxT packed two gates per tensor (values pre-scaled x 2^19)
    d_WxT_if = nc.dram_tensor("WxTif", [2 * IN, HID], bf16, kind="ExternalInput")
    d_WxT_co = nc.dram_tensor("WxTco", [2 * IN, HID], bf16, kind="ExternalInput")
    d_bias = {g: nc.dram_tensor(f"bias{g}", [PT, JT], f32, kind="ExternalInput")
              for g in GATES}
    d_b1 = nc.dram_tensor("b1", [OMID, 1], f32, kind="ExternalInput")
    d_W2T = nc.dram_tensor("W2T", [OMID, 1], bf16, kind="ExternalInput")
    d_sc = nc.dram_tensor("sc", [1, 2], f32, kind="ExternalInput")  # [b2, b_halt]
    d_iota1 = nc.dram_tensor("iota1", [16, BL], i16, kind="ExternalInput")
    d_ones = nc.dram_tensor("ones128", [1, PT], f32, kind="ExternalInput")
    d_out = nc.dram_tensor("out", [1, BL], f32, kind="ExternalOutput")

    with TileContext(nc) as tc:
        if with_lib:
            nc.gpsimd.load_library(library_config.local_scatter)

        with (
            tc.tile_pool(name="const", bufs=1) as cp,
            tc.tile_pool(name="work", bufs=3) as wp,
            tc.tile_pool(name="psg", bufs=1, space="PSUM") as pg,
        ):
            # ---------------- constants / weights -> SBUF ----------------
            def load(d, shape, dt_, tag):
                t = cp.tile(shape, dt_, tag=tag, name=tag)
                nc.sync.dma_start(out=t[:], in_=d.ap())
                return t

            # DMA order matters: everything t=0 needs goes first, the big
            # recurrent weights (needed only from t=1) come last.
            xT_A = load(d_xT, [2 * IN, BL], bf16, "xT_A")
            WxTif = load(d_WxT_if, [2 * IN, HID], bf16, "WxTif")
            WxTco = load(d_WxT_co, [2 * IN, HID], bf16, "WxTco")
            WxT = {"i": WxTif[0:IN, :], "f": WxTif[IN:2 * IN, :],
                   "c": WxTco[0:IN, :], "o": WxTco[IN:2 * IN, :]}
            XOFF = {"i": 0, "c": 0, "f": IN, "o": IN}
            bias = {g: load(d_bias[g], [PT, JT], f32, f"bias{g}") for g in GATES}
            if SWIL:
                W18 = load(d_W18, [PT, KP, 2 * OMID], fp8, "W18")
            else:
                W18 = load(d_W18, [PT, KT, OMID], fp8, "W18")
            Whalt8 = load(d_Whalt8, [PT, KT, 16], fp8, "Whalt8")
            b1 = load(d_b1, [OMID, 1], f32, "b1")
            W2T = load(d_W2T, [OMID, 1], bf16, "W2T")
            sc = load(d_sc, [1, 2], f32, "sc")
            iota1 = load(d_iota1, [16, BL], i16, "iota1")
            ones128 = load(d_ones, [1, PT], f32, "ones128")
            if SWIL:
                Wh8 = {g: load(d_Wh8[g], [PT, KP * JT, 2 * PT], fp8, f"Wh8{g}")
                       for g in GATES}
                x8A = load(d_x8, [PT, 2, BL], fp8, "x8A")
                Wx8 = {g: load(d_Wx8[g], [PT, JT, 2 * PT], fp8, f"Wx8{g}")
                       for g in GATES}
                x8B = cp.tile([PT, 2, BL], fp8, tag="x8B", name="x8B")
                nc.vector.memset(x8B[:, 1, :], 0.0)  # group1 always zero
            else:
                Wh8 = {g: load(d_Wh8[g], [PT, KT, HID], fp8, f"Wh8{g}")
                       for g in GATES}
                x8A = x8B = None

            def rec_mm(psg, g, kp, jt, st8, c0, w, stop):
                """One DoubleRow matmul of the recurrent chain (kp pair)."""
                if SWIL:
                    nc.tensor.matmul(psg[:, :w], Wh8[g][:, kp * JT + jt, :],
                                     st8[:, 2 * kp:2 * kp + 2, c0:c0 + w],
                                     start=False, stop=stop, perf_mode=DRS)
                else:
                    nc.tensor.matmul(psg[:, :w],
                                     Wh8[g][:, 2 * kp:2 * kp + 2,
                                            jt * PT:(jt + 1) * PT],
                                     st8[:, 2 * kp:2 * kp + 2, c0:c0 + w],
                                     start=False, stop=stop, perf_mode=DR)

            def mlp_mm(pm1, st8, kp, c0, w):
                if SWIL:
                    nc.tensor.matmul(pm1[:, :w], W18[:, kp, :],
                                     st8[:, 2 * kp:2 * kp + 2, c0:c0 + w],
                                     start=(kp == 0), stop=(kp == KP - 1),
                                     perf_mode=DRS)
                else:
                    nc.tensor.matmul(pm1[:, :w], W18[:, 2 * kp:2 * kp + 2, :],
                                     st8[:, 2 * kp:2 * kp + 2, c0:c0 + w],
                                     start=(kp == 0), stop=(kp == KP - 1),
                                     perf_mode=DR)

            def halt_mm(psh, st8, kp, c0, w):
                nc.tensor.matmul(psh[:, :w], Whalt8[:, 2 * kp:2 * kp + 2, 0:1],
                                 st8[:, 2 * kp:2 * kp + 2, c0:c0 + w],
                                 start=(kp == 0), stop=(kp == KP - 1),
                                 perf_mode=DR)

            stA = [cp.tile([PT, BL], bf16, tag=f"stA{kt}", name=f"stA{kt}")
                   for kt in range(KT)]
            clA = [cp.tile([PT, BL], bf16, tag=f"clA{kt}", name=f"clA{kt}")
                   for kt in range(KT)]
            # fp8 state in DoubleRow k-major layout: [p, k, n] = state.T[k*128+p, n]*SS
            st8A = cp.tile([PT, KT, BL], fp8, tag="st8A", name="st8A")
            st8B = cp.tile([PT, KT, BL], fp8, tag="st8B", name="st8B")
            # compaction daisy-chains into the (then dead) A tiles: the compact
            # buffers are [tmp, stA[0..6]]; xT compacts into stA[7]
            tmp_st = cp.tile([PT, BL], bf16, tag="tmp_st", name="tmp_st")
            tmp_cl = cp.tile([PT, BL], bf16, tag="tmp_cl", name="tmp_cl")
            stB = [tmp_st] + stA[:KT - 1]
            clB = [tmp_cl] + clA[:KT - 1]
            # dedicated compact-x destination so its scatter can run FIRST
            # (t2's first matmul chain starts from x)
            xT_B = cp.tile([2 * IN, BL], bf16, tag="xT_B", name="xT_B")

            # per-row scalars: separate [1, BL] tiles (engine tensor-tensor ops
            # require all SBUF operands at the same start partition)
            ROWS = {nm: cp.tile([1, BL], f32, tag=f"rv_{nm}", name=f"rv_{nm}")
                    for nm in ["act", "p", "acc", "ctr", "omp", "fin", "hw", "h",
                               "out"]}

            def row(nm, c0=0, c1=BL):
                return ROWS[nm][0:1, c0:c1]

            cb16 = cp.tile([16, BL], bf16, tag="cb16")    # contrib scatter staging
            sct16 = cp.tile([16, BL], bf16, tag="sct16")  # 16-channel scatter dst
            phi16 = cp.tile([16, BL], bf16, tag="phi16")  # p_sum hi part
            plo16 = cp.tile([16, BL], bf16, tag="plo16")  # p_sum lo part
            orig16 = cp.tile([16, BL], i16, tag="orig16")  # orig row ids (-1 invalid)
            # only row 0 of the 16-channel staging tiles carries data; the
            # scatters read all 16 rows, so initialize them once
            nc.vector.memset(cb16[:], 0.0)
            nc.vector.memset(phi16[:], 0.0)
            nc.vector.memset(plo16[:], 0.0)
            cum_row = cp.tile([1, BL], f32, tag="cum_row")
            dst_row = cp.tile([1, BL], f32, tag="dst_row")
            idx128 = cp.tile([PT, BL], i16, tag="idx128")

            # ---------------- shared pieces ----------------
            def mlp_and_halt(st8, c0, w):
                """MLP head + halt logit for batch cols [c0, c0+w).
                fp8 DoubleRow chains over the fp8 state.
                Writes sigmoid outputs into rows out / h."""
                pm1 = pg.tile([OMID, NBLK], f32, tag="psgi0", name="pm1")
                for kp in range(KP):
                    mlp_mm(pm1, st8, kp, c0, w)
                relu1 = wp.tile([OMID, NBLK], bf16, tag="relu1", name="relu1")
                nc.scalar.activation(relu1[:, :w], pm1[:, :w], AF.Relu,
                                     bias=b1[:, 0:1], scale=SCL)
                ps2 = pg.tile([1, NBLK], f32, tag="psgf0", name="ps2")
                nc.tensor.matmul(ps2[:, :w], W2T[:], relu1[:, :w],
                                 start=True, stop=True)
                psh = pg.tile([1, NBLK], f32, tag="psgc0", name="psh")
                for kp in range(KP):
                    halt_mm(psh, st8, kp, c0, w)
                nc.scalar.activation(row("out", c0, c0 + w), ps2[:, :w], AF.Sigmoid,
                                     bias=sc[0:1, 0:1], scale=1.0)
                nc.scalar.activation(row("h", c0, c0 + w), psh[:, :w], AF.Sigmoid,
                                     bias=sc[0:1, 1:2], scale=SCL)

            def halt_math(c0, w, is_last):
                """Halting update on batch cols [c0, c0+w). Produces contrib row."""
                c1 = c0 + w
                p, a = row("p", c0, c1), row("act", c0, c1)
                omp, fin = row("omp", c0, c1), row("fin", c0, c1)
                hw_, h = row("hw", c0, c1), row("h", c0, c1)
                out, ctr = row("out", c0, c1), row("ctr", c0, c1)
                # omp = 1 - p_sum (old)
                nc.vector.tensor_scalar(out=omp, in0=p, scalar1=-1.0, scalar2=1.0,
                                        op0=ALU.mult, op1=ALU.add)
                # p_sum += h  (now p_new)
                nc.vector.tensor_add(out=p, in0=p, in1=h)
                if is_last:
                    nc.vector.tensor_copy(out=fin, in_=a)
                else:
                    nc.vector.tensor_scalar(out=fin, in0=p, scalar1=1.0 - EPS,
                                            scalar2=None, op0=ALU.is_ge)
                    nc.vector.tensor_mul(out=fin, in0=fin, in1=a)
                # hw = fin ? omp : h  ==  (omp - h)*fin + h   (then mask by act)
                nc.vector.tensor_sub(out=hw_, in0=omp, in1=h)
                nc.vector.tensor_mul(out=hw_, in0=hw_, in1=fin)
                nc.vector.tensor_add(out=hw_, in0=hw_, in1=h)
                nc.vector.tensor_mul(out=hw_, in0=hw_, in1=a)
                nc.vector.tensor_mul(out=ctr, in0=out, in1=hw_)
                nc.vector.tensor_sub(out=a, in0=a, in1=fin)

            def gates_block(t, st, cl, st8, xT, x8, nbs):
                """Gate matmuls + cell/state update for the batch blocks in nbs
                (list of (c0, w)). With >1 block the weight tile loaded for a
                (g, kp, jt) is streamed over every block before moving on,
                amortizing DoubleRow's (FWL-less) weight loads.

                Two phases: (A) per jt: matmul chains, ACT evictions and the
                DVE cell math; (B) tanh(cell) + state mul for all jt."""
                glist = "ico" if t == 0 else GATES
                o_tiles = []
                for jt in range(JT):
                    gt = {g: [] for g in glist}
                    for g in glist:
                        psl = []
                        for bi, (c0, w) in enumerate(nbs):
                            tag = f"psg{g}{bi if len(nbs) > 1 else jt % 2}"
                            psg = pg.tile([PT, NBLK], f32, tag=tag, name=tag)
                            if t > 0 and SWIL:
                                # x-projection as fp8 DRS too (dup trick) so the
                                # whole chain stays in DoubleRow mode
                                nc.tensor.matmul(psg[:, :w], Wx8[g][:, jt, :],
                                                 x8[:, :, c0:c0 + w],
                                                 start=True, stop=False,
                                                 perf_mode=DRS)
                            else:
                                nc.tensor.matmul(psg[:, :w],
                                                 WxT[g][:, jt * PT:(jt + 1) * PT],
                                                 xT[XOFF[g]:XOFF[g] + IN, c0:c0 + w],
                                                 start=True, stop=(t == 0))
                            psl.append(psg)
                        if t > 0:
                            for kp in range(KP):
                                for bi, (c0, w) in enumerate(nbs):
                                    rec_mm(psl[bi], g, kp, jt, st8, c0, w,
                                           stop=(kp == KP - 1))
                        for bi, (c0, w) in enumerate(nbs):
                            gtile = wp.tile([PT, NBLK], bf16, tag=f"g{g}{bi}",
                                            name=f"g{g}{bi}",
                                            bufs=(JT + 1 if g == "o" else 2))
                            nc.scalar.activation(gtile[:, :w], psl[bi][:, :w],
                                                 AF.Tanh if g == "c" else AF.Sigmoid,
                                                 bias=bias[g][:, jt:jt + 1],
                                                 scale=SCL)
                            gt[g].append(gtile)
                    for bi, (c0, w) in enumerate(nbs):
                        if t == 0:
                            # cell = i*c
                            nc.vector.tensor_mul(cl[jt][:, c0:c0 + w],
                                                 gt["i"][bi][:, :w],
                                                 gt["c"][bi][:, :w])
                        else:
                            # i *= c ; f *= cell ; cell = i + f  (in gate tiles)
                            nc.vector.tensor_mul(gt["i"][bi][:, :w],
                                                 gt["i"][bi][:, :w],
                                                 gt["c"][bi][:, :w])
                            nc.vector.tensor_mul(gt["f"][bi][:, :w],
                                                 gt["f"][bi][:, :w],
                                                 cl[jt][:, c0:c0 + w])
                            nc.vector.tensor_add(cl[jt][:, c0:c0 + w],
                                                 gt["i"][bi][:, :w],
                                                 gt["f"][bi][:, :w])
                    o_tiles.append(gt["o"])
                for jt in range(JT):
                    for bi, (c0, w) in enumerate(nbs):
                        # state = o * tanh(cell); also refresh the fp8 copy
                        th = wp.tile([PT, NBLK], bf16, tag="th", name="th")
                        nc.scalar.activation(th[:, :w], cl[jt][:, c0:c0 + w],
                                             AF.Tanh)
                        nc.vector.tensor_mul(st[jt][:, c0:c0 + w],
                                             o_tiles[jt][bi][:, :w], th[:, :w])
                        nc.vector.tensor_scalar(out=st8[:, jt, c0:c0 + w],
                                                in0=st[jt][:, c0:c0 + w],
                                                scalar1=SS, scalar2=None,
                                                op0=ALU.mult)

            def acc_add_direct(c0, w):
                c1 = c0 + w
                nc.vector.tensor_add(out=row("acc", c0, c1),
                                     in0=row("acc", c0, c1),
                                     in1=row("ctr", c0, c1))

            # ============ main body (repeatable for slope timing) ============
            REP = [0]

            def main_body():
              nc.vector.memset(row("p"), 0.0)
              nc.vector.memset(row("act"), 1.0)
              nc.vector.memset(row("acc"), 0.0)
              def dense_iter(t):
                  # all gate matmuls first so the PE stream never waits on the
                  # DVE state-update epilogue of the previous batch block
                  if opts["pair"]:
                      gates_block(t, stA, clA, st8A, xT_A, x8A,
                                  [(0, NBLK), (NBLK, NBLK)])
                      for nb in range(NBLKS):
                          mlp_and_halt(st8A, nb * NBLK, NBLK)
                          halt_math(nb * NBLK, NBLK, is_last=False)
                          acc_add_direct(nb * NBLK, NBLK)
                  elif opts["gates_first"]:
                      for nb in range(NBLKS):
                          gates_block(t, stA, clA, st8A, xT_A, x8A,
                                      [(nb * NBLK, NBLK)])
                      for nb in range(NBLKS):
                          mlp_and_halt(st8A, nb * NBLK, NBLK)
                          halt_math(nb * NBLK, NBLK, is_last=False)
                          acc_add_direct(nb * NBLK, NBLK)
                  else:
                      for nb in range(NBLKS):
                          gates_block(t, stA, clA, st8A, xT_A, x8A,
                                      [(nb * NBLK, NBLK)])
                          mlp_and_halt(st8A, nb * NBLK, NBLK)
                          halt_math(nb * NBLK, NBLK, is_last=False)
                          acc_add_direct(nb * NBLK, NBLK)

              # ------------- t = 0 (state==0: x-projection only) -------------
              dense_iter(0)

              # ---------------- t = 1 (dense) ----------------
              if upto == 't0':
                  return
              dense_iter(1)

              if upto == 't1':
                  return

              # total active count -> one register on every engine (fresh
              # tiles per call: the raw reg_load reads are not fully
              # WAR-tracked by Tile)
              def count_total(tagix):
                  cntf = cp.tile([1, 8], f32, tag=f"cntf{REP[0]}_{tagix}", name=f"cntf{REP[0]}_{tagix}")
                  cnti = cp.tile([1, 8], i32, tag=f"cnti{REP[0]}_{tagix}", name=f"cnti{REP[0]}_{tagix}")
                  nc.vector.reduce_sum(out=cntf[0:1, 0:1], in_=row("act"),
                                       axis=mybir.AxisListType.X)
                  nc.vector.tensor_copy(out=cnti[0:1, 0:1], in_=cntf[0:1, 0:1])
                  return nc.values_load(cnti[0:1, 0:1], min_val=0, max_val=BL,
                                        skip_runtime_bounds_check=True)

              def compaction_idx_half(nb):
                  # prefix-sum of act for one 512-col half (chained via the
                  # previous half's last element); dest = cumsum*act - 1.
                  # Half 0 only depends on t1 block 0's halting update, so it
                  # hides behind t1 block 1's matmuls.
                  c0, c1 = nb * NBLK, (nb + 1) * NBLK
                  nc.vector.tensor_tensor_scan(
                      out=cum_row[0:1, c0:c1], data0=row("act", c0, c1),
                      data1=row("act", c0, c1),
                      initial=0.0 if nb == 0 else cum_row[0:1, c0 - 1:c0],
                      op0=ALU.add, op1=ALU.max)
                  dst = dst_row[0:1, c0:c1]
                  nc.vector.tensor_mul(out=dst, in0=cum_row[0:1, c0:c1],
                                       in1=row("act", c0, c1))
                  nc.vector.tensor_scalar_add(out=dst, in0=dst, scalar1=-1.0)
                  # broadcast dest to 128 partitions (ones outer product; f32
                  # matmul is exact for these small integers)
                  ptag = "psgi0" if nb == 0 else "psgf0"
                  pb = pg.tile([PT, NBLK], f32, tag=ptag, name=f"pb{nb}")
                  nc.tensor.matmul(pb[:], ones128[:], dst, start=True, stop=True)
                  nc.vector.tensor_copy(out=idx128[:, c0:c1], in_=pb[:])

              def compaction():
                  # compact x first (t2's chains start from it), then state in
                  # chain order (dst k is src k+1), then cell; after each state
                  # tile lands, refresh the fp8 DoubleRow copy (full width so
                  # guarded t>=3 blocks never read uninitialized fp8)
                  nc.gpsimd.local_scatter(xT_B[:], xT_A[:], idx128[:],
                                          2 * IN, BL, BL)
                  if SWIL:
                      nc.vector.tensor_scalar(out=x8B[:, 0, :], in0=xT_B[:],
                                              scalar1=XSC, scalar2=None,
                                              op0=ALU.mult)
                  for kt in range(KT):
                      nc.gpsimd.local_scatter(stB[kt][:], stA[kt][:], idx128[:],
                                              PT, BL, BL)
                      nc.vector.tensor_scalar(out=st8B[:, kt, :],
                                              in0=stB[kt][:],
                                              scalar1=SS, scalar2=None,
                                              op0=ALU.mult)
                  for kt in range(KT):
                      nc.gpsimd.local_scatter(clB[kt][:], clA[kt][:], idx128[:],
                                              PT, BL, BL)
                  # compact p_sum as bf16 hi+lo split (exact to ~2^-17)
                  nc.vector.tensor_copy(out=phi16[0:1, :], in_=row("p"))
                  nc.vector.tensor_sub(out=plo16[0:1, :], in0=row("p"),
                                       in1=phi16[0:1, :])
                  nc.gpsimd.local_scatter(sct16[:], phi16[:], idx128[0:16, :],
                                          16, BL, BL)
                  nc.vector.tensor_copy(out=phi16[0:1, :], in_=sct16[0:1, :])
                  nc.gpsimd.local_scatter(sct16[:], plo16[:], idx128[0:16, :],
                                          16, BL, BL)
                  nc.vector.tensor_add(out=row("p"), in0=phi16[0:1, :],
                                       in1=sct16[0:1, :])
                  # compact original row ids (1-based -> 0-based; empty -> -1)
                  nc.gpsimd.local_scatter(orig16[:], iota1[:], idx128[0:16, :],
                                          16, BL, BL)
                  nc.vector.tensor_scalar(out=orig16[:], in0=orig16[:], scalar1=1,
                                          scalar2=None, op0=ALU.subtract)
                  # act := (slot occupied) in compact space
                  nc.vector.tensor_scalar(out=row("act"), in0=orig16[0:1, :],
                                          scalar1=0, scalar2=None, op0=ALU.is_ge)

              if upto == 'compact':
                  compaction_idx_half(0)
                  compaction_idx_half(1)
                  compaction()
                  return

              def block_work(t, c0, w):
                  gates_block(t, stB, clB, st8B, xT_B, x8B, [(c0, w)])
                  mlp_and_halt(st8B, c0, w)
                  halt_math(c0, w, is_last=(t == MAX_ITER - 1))
                  nc.vector.tensor_copy(out=cb16[0:1, c0:c0 + w],
                                        in_=row("ctr", c0, c0 + w))

              def acc_scatter():
                  # scatter contributions back to original row order
                  nc.gpsimd.local_scatter(sct16[:], cb16[:], orig16[:],
                                          16, BL, BL)
                  nc.vector.tensor_add(out=row("acc"), in0=row("acc"),
                                       in1=sct16[0:1, :])

              # timing variant: guard-free tail (correct only when every row
              # halts by t=2 in <=384 compact columns -- measurement only)
              if upto == 'noif':
                  compaction_idx_half(0)
                  compaction_idx_half(1)
                  compaction()
                  nc.vector.memset(cb16[0:1, :], 0.0)
                  block_work(2, 0, 384)
                  acc_scatter()
                  return

              # ---------------- t = 2..7 (compact space, fully guarded) -------
              def late_iter(t, cnt):
                  pf_skip = False if opts["ft_hints"] else None
                  with tc.If(cnt > 0, preferred_fallthrough_block=pf_skip):
                      nc.vector.memset(cb16[0:1, :], 0.0)
                      # actives may sit anywhere in compact space at t>=3 (no
                      # recompaction); run both halves under the one guard
                      block_work(t, 0, NBLK)
                      block_work(t, NBLK, NBLK)
                      acc_scatter()
                      if t < MAX_ITER - 1:
                          late_iter(t + 1, count_total(t))

              # t=2: compaction runs inside the guard (skipped if all rows
              # halted) so the scatters pipeline with t2's first matmul chains;
              # in compact space actives are contiguous, so block0 needs no
              # guard and later blocks guard on n2 thresholds.
              compaction_idx_half(0)
              compaction_idx_half(1)
              n2 = count_total(1)
              pf2 = True if opts["ft_hints"] else None
              pf_skip2 = False if opts["ft_hints"] else None
              # t2 block plan: compact actives are contiguous from col 0, so
              # use finer 384/384/256 blocks, each guarded by n2 thresholds
              with tc.If(n2 > 0, preferred_fallthrough_block=pf2):
                  compaction()
                  nc.vector.memset(cb16[0:1, :], 0.0)
                  block_work(2, 0, 384)
                  with tc.If(n2 > 384, preferred_fallthrough_block=pf_skip2):
                      block_work(2, 384, 384)
                  with tc.If(n2 > 768, preferred_fallthrough_block=pf_skip2):
                      block_work(2, 768, 256)
                  acc_scatter()
                  late_iter(3, count_total(2))

            for _rep in range(reps):
                REP[0] = _rep
                main_body()

            # ---------------- output ----------------
            nc.sync.dma_start(out=d_out.ap()[:, :], in_=row("acc"))

    nc.compile()
    return nc


def _interleave_dr(WT, m):
    """WT: [HID, m] k-major -> [128, KP, 2m] fp8, SwInterleave layout:
    [p, kp, 2j] = WT[2kp*128+p, m-1-j]*WS, [p, kp, 2j+1] = WT[(2kp+1)*128+p, m-1-j]."""
    f8 = ml_dtypes.float8_e4m3
    a = (WT.astype(np.float32) * WS).reshape(KP, 2, PT, m)  # [kp, i, p, j]
    out = np.zeros((PT, KP, 2 * m), np.float32)
    out[:, :, 0::2] = a[:, 0].transpose(1, 0, 2)[:, :, ::-1]
    out[:, :, 1::2] = a[:, 1].transpose(1, 0, 2)[:, :, ::-1]
    return np.clip(out, -240.0, 240.0).astype(f8)


def _prep_inputs(x, Wxi, bxi, Whi, bhi, Wxf, bxf, Whf, bhf, Wxc, bxc, Whc, bhc,
                 Wxo, bxo, Who, bho, W_halt, b_halt, W1, b1, W2, b2):
    bf = ml_dtypes.bfloat16
    f8 = ml_dtypes.float8_e4m3
    XS = np.float32(WS * SS)  # x-projection weight pre-scale (2^19)

    def to_dr(WT, m):
        # WT: [HID, m] (k-major) -> [128, KT, m] fp8 with [p, k, j] = WT[k*128+p, j]*WS
        a = (WT.astype(np.float32) * WS).reshape(KT, PT, m).transpose(1, 0, 2)
        return np.clip(a, -240.0, 240.0).astype(f8)

    gw = {"i": (Wxi, bxi, Whi, bhi), "f": (Wxf, bxf, Whf, bhf),
          "c": (Wxc, bxc, Whc, bhc), "o": (Wxo, bxo, Who, bho)}
    shared = {}
    for g, (Wx, bx, Wh, bh) in gw.items():
        WhT = Wh.T
        shared[f"Wh8{g}"] = np.ascontiguousarray(to_dr(WhT, HID))
        # SwInterleave layout is per (kp, jt) block of 128 output cols
        swi = np.zeros((PT, KP * JT, 2 * PT), ml_dtypes.float8_e4m3)
        for jt in range(JT):
            blk = _interleave_dr(WhT[:, jt * PT:(jt + 1) * PT], PT)  # [128,KP,256]
            for kp in range(KP):
                swi[:, kp * JT + jt, :] = blk[:, kp, :]
        shared[f"Wh8s{g}"] = np.ascontiguousarray(swi)
        # fp8 SwInterleave x-weights (dup trick): group0 = Wx.T[p%64]*WXS, group1 = 0
        wxd = np.zeros((PT, HID), np.float32)
        wxt = Wx.astype(np.float32).T * WXS          # [IN, HID]
        wxd[0:IN] = wxt
        wxd[IN:2 * IN] = wxt
        wx8 = np.zeros((PT, JT, 2 * PT), np.float32)
        for jt in range(JT):
            wx8[:, jt, 0::2] = wxd[:, jt * PT:(jt + 1) * PT][:, ::-1]
        shared[f"Wx8s{g}"] = np.ascontiguousarray(
            np.clip(wx8, -240.0, 240.0).astype(f8))
        shared[f"bias{g}"] = np.ascontiguousarray(
            (bx + bh).astype(np.float32).reshape(JT, PT).T)
    shared["WxTif"] = np.ascontiguousarray(
        np.concatenate([gw["i"][0].T, gw["f"][0].T], axis=0)
        .astype(np.float32) * XS).astype(bf)
    shared["WxTco"] = np.ascontiguousarray(
        np.concatenate([gw["c"][0].T, gw["o"][0].T], axis=0)
        .astype(np.float32) * XS).astype(bf)
    shared["W18"] = np.ascontiguousarray(to_dr(W1.T, OMID))
    shared["W18s"] = np.ascontiguousarray(_interleave_dr(W1.T, OMID))
    shared["b1"] = b1.astype(np.float32).reshape(OMID, 1)
    shared["W2T"] = np.ascontiguousarray(W2.T).astype(bf)
    whalt = np.zeros((HID, 16), np.float32)
    whalt[:, 0] = W_halt.astype(np.float32).reshape(HID)
    shared["Whalt8"] = np.ascontiguousarray(to_dr(whalt, 16))
    # halt SwInterleave: m=1 -> per kp just [A_0, B_0] in cols 0:2 (16-wide pad)
    whs = np.zeros((PT, KP, 16), np.float32)
    wh = (W_halt.astype(np.float32).reshape(KP, 2, PT) * WS)
    whs[:, :, 0] = wh[:, 0].T
    whs[:, :, 1] = wh[:, 1].T
    shared["Whalt8s"] = np.ascontiguousarray(whs.astype(f8))
    shared["sc"] = np.array([[b2[0], b_halt[0]]], dtype=np.float32)
    shared["iota1"] = np.tile(np.arange(1, BL + 1, dtype=np.int16), (16, 1))
    shared["ones128"] = np.ones((1, PT), dtype=np.float32)

    in_maps = []
    for c in range(NCORES):
        m = dict(shared)
        xs = x[c * BL:(c + 1) * BL].astype(np.float32)
        xt = np.ascontiguousarray(xs.T).astype(bf)
        m["xT"] = np.concatenate([xt, xt], axis=0)
        x8 = np.zeros((PT, 2, BL), np.float32)
        x8[0:IN, 0] = xs.T * XSC
        x8[IN:2 * IN, 0] = xs.T * XSC
        m["x8"] = np.ascontiguousarray(np.clip(x8, -240.0, 240.0).astype(f8))
        in_maps.append(m)
    return in_maps


def kernel(**inputs):
    from concourse.bass_utils import run_bass_kernel_spmd

    if "nc" not in _CACHE:
        _CACHE["nc"] = _build_nc()
    nc = _CACHE["nc"]

    in_maps = _prep_inputs(**{k: np.asarray(v) for k, v in inputs.items()})
    res = run_bass_kernel_spmd(nc, in_maps, core_ids=list(range(NCORES)))
    out = np.concatenate([res.results[c]["out"][0] for c in range(NCORES)])
    return out.reshape(BATCH, 1).astype(np.float32)


# revision 19
# speedup vs baseline: 6.2124x; 6.2124x over previous
"""ACT-LSTM (adaptive computation time) Bass/Tile kernel for 8 TRN2 NeuronCores.

Model (per batch row, up to 8 iterations):
    4 LSTM gates:  g = act(x @ Wx_g.T + bx_g + state @ Wh_g.T + bh_g)
    cell  = f*cell + i*c ; state = o*tanh(cell)
    out   = sigmoid(relu(state @ W1.T + b1) @ W2.T + b2)
    h     = sigmoid(state @ W_halt.T + b_halt); rows halt when cumsum(h) >= 1-eps
    final = sum_t out_t * halt_weight_t

Strategy:
  - Data-parallel: batch 8192 sharded to 8 cores (1024 rows each). Weights replicated.
  - Transposed layout everywhere: hidden dim on SBUF partitions, batch on the free
    dim, so the recurrent matmul needs no per-iteration transposes.
  - fp8(e4m3) DoubleRowSwInterleave matmuls for everything contracting over
    HID=1024 (the 4 recurrent gate projections, the W1 MLP projection, the halt
    projection): weights pre-scaled x4096 and pre-interleaved on host (pairs
    interleaved, columns reversed) so LDWEIGHTS reads contiguously; state stored
    fp8 x128 in [128, 8, BL] k-major layout. The x-projection stays bf16
    (weights pre-scaled x2^19 so the whole PSUM chain carries one scale,
    removed by the activation's scale=2^-19). Halting decisions are
    self-correcting (flipped rows carry ~1-p_sum ~ 1e-3 of mass), so fp8 noise
    is benign: measured rel err ~6e-4.
  - Weight-paired dense iterations: both 512-col batch blocks are streamed per
    loaded weight tile, amortizing DoubleRow's (FWL-less) weight loads.
  - t=0 specialization: state==0, so gates need only the x-projection.
  - Ragged-sequence exploitation: after t=1 the still-active rows are compacted
    (stream compaction via GPSIMD local_scatter with prefix-sum indices), so
    iteration 2 runs on <=512 columns instead of 1024. Iterations 3..7 are guarded
    by data-dependent Ifs and are skipped entirely once every row has halted.
  - Per-batch-row scalars (p_sum/act/acc/...) are packed on separate [1, BL] rows.
"""

import sys

sys.path.insert(0, "/opt/trn_rl_repo")

import numpy as np
import ml_dtypes

BATCH, IN, HID, OMID = 8192, 64, 1024, 128
MAX_ITER = 8
EPS = 1e-3
NCORES = 8
BL = BATCH // NCORES          # 1024 rows per core
NBLK = 512                    # matmul moving-dim (batch) block
NBLKS = BL // NBLK            # 2
PT = 128
KT = HID // PT                # 8 contraction tiles
JT = HID // PT                # 8 output-row tiles
KP = KT // 2                  # 4 DoubleRow contraction pairs

WS = 4096.0                   # fp8 weight scale
SS = 128.0                    # fp8 state scale
SCL = 1.0 / (WS * SS)         # 2^-19: descale applied in activations

_CACHE: dict = {}


def _build_nc(reps=1, upto='full', with_lib=True, opts=None):
    opts = dict(dict(gates_first=True, ft_hints=True, pair=True, swil=True,
                     no_mlp=False),
                **(opts or {}))
    import concourse.mybir as mybir
    from concourse import bacc, library_config
    from concourse.tile import TileContext

    f32 = mybir.dt.float32
    bf16 = mybir.dt.bfloat16
    fp8 = mybir.dt.float8e4
    i16 = mybir.dt.int16
    i32 = mybir.dt.int32
    AF = mybir.ActivationFunctionType
    ALU = mybir.AluOpType
    DR = mybir.MatmulPerfMode.DoubleRow
    DRS = mybir.MatmulPerfMode.DoubleRowSwInterleave
    SWIL = opts["swil"]

    nc = bacc.Bacc("TRN2", target_bir_lowering=False, debug=False,
                   enable_asserts=False)

    GATES = "ifco"
    # ---- DRAM parameters (all pre-transposed / pre-cast on host) ----
    d_xT = nc.dram_tensor("xT", [2 * IN, BL], bf16, kind="ExternalInput")
    if SWIL:
        # software-interleaved DoubleRow weights: [p, kp*JT+jt, 2j] =
        # Wh.T[2kp*128+p, jt*128+127-j]*WS, [.., 2j+1] = Wh.T[(2kp+1)*128+p, ..]
        d_Wh8 = {g: nc.dram_tensor(f"Wh8s{g}", [PT, KP * JT, 2 * PT], fp8,
                                   kind="ExternalInput") for g in GATES}
        d_W18 = nc.dram_tensor("W18s", [PT, KP, 2 * OMID], fp8,
                               kind="ExternalInput")
        # halt stays chunked-DR: a 2-active-col SwInterleave LDWEIGHTS fails
        # walrus's s3_lw_valid_num_active_cols check
        d_Whalt8 = nc.dram_tensor("Whalt8", [PT, KT, 16], fp8,
                                  kind="ExternalInput")
    else:
        # chunked DoubleRow weights: [p, k, j] = Wh.T[k*128+p, j] * WS
        d_Wh8 = {g: nc.dram_tensor(f"Wh8{g}", [PT, KT, HID], fp8,
                                   kind="ExternalInput") for g in GATES}
        d_W18 = nc.dram_tensor("W18", [PT, KT, OMID], fp8, kind="ExternalInput")
        d_Whalt8 = nc.dram_tensor("Whalt8", [PT, KT, 16], fp8,
                                  kind="ExternalInput")
    # WxT packed two gates per tensor (values pre-scaled x 2^19)
    d_WxT_if = nc.dram_tensor("WxTif", [2 * IN, HID], bf16, kind="ExternalInput")
    d_WxT_co = nc.dram_tensor("WxTco", [2 * IN, HID], bf16, kind="ExternalInput")
    d_bias = {g: nc.dram_tensor(f"bias{g}", [PT, JT], f32, kind="ExternalInput")
              for g in GATES}
    d_b1 = nc.dram_tensor("b1", [OMID, 1], f32, kind="ExternalInput")
    d_W2T = nc.dram_tensor("W2T", [OMID, 1], bf16, kind="ExternalInput")
    d_sc = nc.dram_tensor("sc", [1, 2], f32, kind="ExternalInput")  # [b2, b_halt]
    d_iota1 = nc.dram_tensor("iota1", [16, BL], i16, kind="ExternalInput")
    d_ones = nc.dram_tensor("ones128", [1, PT], f32, kind="ExternalInput")
    d_out = nc.dram_tensor("out", [1, BL], f32, kind="ExternalOutput")

    with TileContext(nc) as tc:
        if with_lib:
            nc.gpsimd.load_library(library_config.local_scatter)

        with (
            tc.tile_pool(name="const", bufs=1) as cp,
            tc.tile_pool(name="work", bufs=3) as wp,
            tc.tile_pool(name="psg", bufs=1, space="PSUM") as pg,
        ):
            # ---------------- constants / weights -> SBUF ----------------
            def load(d, shape, dt_, tag):
                t = cp.tile(shape, dt_, tag=tag, name=tag)
                nc.sync.dma_start(out=t[:], in_=d.ap())
                return t

            # DMA order matters: everything t=0 needs goes first, the big
            # recurrent weights (needed only from t=1) come last.
            xT_A = load(d_xT, [2 * IN, BL], bf16, "xT_A")
            WxTif = load(d_WxT_if, [2 * IN, HID], bf16, "WxTif")
            WxTco = load(d_WxT_co, [2 * IN, HID], bf16, "WxTco")
            WxT = {"i": WxTif[0:IN, :], "f": WxTif[IN:2 * IN, :],
                   "c": WxTco[0:IN, :], "o": WxTco[IN:2 * IN, :]}
            XOFF = {"i": 0, "c": 0, "f": IN, "o": IN}
            bias = {g: load(d_bias[g], [PT, JT], f32, f"bias{g}") for g in GATES}
            if SWIL:
                W18 = load(d_W18, [PT, KP, 2 * OMID], fp8, "W18")
            else:
                W18 = load(d_W18, [PT, KT, OMID], fp8, "W18")
            Whalt8 = load(d_Whalt8, [PT, KT, 16], fp8, "Whalt8")
            b1 = load(d_b1, [OMID, 1], f32, "b1")
            W2T = load(d_W2T, [OMID, 1], bf16, "W2T")
            sc = load(d_sc, [1, 2], f32, "sc")
            iota1 = load(d_iota1, [16, BL], i16, "iota1")
            ones128 = load(d_ones, [1, PT], f32, "ones128")
            if SWIL:
                Wh8 = {g: load(d_Wh8[g], [PT, KP * JT, 2 * PT], fp8, f"Wh8{g}")
                       for g in GATES}
            else:
                Wh8 = {g: load(d_Wh8[g], [PT, KT, HID], fp8, f"Wh8{g}")
                       for g in GATES}

            def rec_mm(psg, g, kp, jt, st8, c0, w, stop):
                """One DoubleRow matmul of the recurrent chain (kp pair)."""
                if SWIL:
                    nc.tensor.matmul(psg[:, :w], Wh8[g][:, kp * JT + jt, :],
                                     st8[:, 2 * kp:2 * kp + 2, c0:c0 + w],
                                     start=False, stop=stop, perf_mode=DRS)
                else:
                    nc.tensor.matmul(psg[:, :w],
                                     Wh8[g][:, 2 * kp:2 * kp + 2,
                                            jt * PT:(jt + 1) * PT],
                                     st8[:, 2 * kp:2 * kp + 2, c0:c0 + w],
                                     start=False, stop=stop, perf_mode=DR)

            def mlp_mm(pm1, st8, kp, c0, w):
                if SWIL:
                    nc.tensor.matmul(pm1[:, :w], W18[:, kp, :],
                                     st8[:, 2 * kp:2 * kp + 2, c0:c0 + w],
                                     start=(kp == 0), stop=(kp == KP - 1),
                                     perf_mode=DRS)
                else:
                    nc.tensor.matmul(pm1[:, :w], W18[:, 2 * kp:2 * kp + 2, :],
                                     st8[:, 2 * kp:2 * kp + 2, c0:c0 + w],
                                     start=(kp == 0), stop=(kp == KP - 1),
                                     perf_mode=DR)

            def halt_mm(psh, st8, kp, c0, w):
                nc.tensor.matmul(psh[:, :w], Whalt8[:, 2 * kp:2 * kp + 2, 0:1],
                                 st8[:, 2 * kp:2 * kp + 2, c0:c0 + w],
                                 start=(kp == 0), stop=(kp == KP - 1),
                                 perf_mode=DR)

            stA = [cp.tile([PT, BL], bf16, tag=f"stA{kt}", name=f"stA{kt}")
                   for kt in range(KT)]
            clA = [cp.tile([PT, BL], bf16, tag=f"clA{kt}", name=f"clA{kt}")
                   for kt in range(KT)]
            # fp8 state in DoubleRow k-major layout: [p, k, n] = state.T[k*128+p, n]*SS
            st8A = cp.tile([PT, KT, BL], fp8, tag="st8A", name="st8A")
            st8B = cp.tile([PT, KT, BL], fp8, tag="st8B", name="st8B")
            # compaction daisy-chains into the (then dead) A tiles: the compact
            # buffers are [tmp, stA[0..6]]; xT compacts into stA[7]
            tmp_st = cp.tile([PT, BL], bf16, tag="tmp_st", name="tmp_st")
            tmp_cl = cp.tile([PT, BL], bf16, tag="tmp_cl", name="tmp_cl")
            stB = [tmp_st] + stA[:KT - 1]
            clB = [tmp_cl] + clA[:KT - 1]
            # dedicated compact-x destination so its scatter can run FIRST
            # (t2's first matmul chain starts from x)
            xT_B = cp.tile([2 * IN, BL], bf16, tag="xT_B", name="xT_B")

            # per-row scalars: separate [1, BL] tiles (engine tensor-tensor ops
            # require all SBUF operands at the same start partition)
            ROWS = {nm: cp.tile([1, BL], f32, tag=f"rv_{nm}", name=f"rv_{nm}")
                    for nm in ["act", "p", "acc", "ctr", "omp", "fin", "hw", "h",
                               "out"]}

            def row(nm, c0=0, c1=BL):
                return ROWS[nm][0:1, c0:c1]

            cb16 = cp.tile([16, BL], bf16, tag="cb16")    # contrib scatter staging
            sct16 = cp.tile([16, BL], bf16, tag="sct16")  # 16-channel scatter dst
            phi16 = cp.tile([16, BL], bf16, tag="phi16")  # p_sum hi part
            plo16 = cp.tile([16, BL], bf16, tag="plo16")  # p_sum lo part
            orig16 = cp.tile([16, BL], i16, tag="orig16")  # orig row ids (-1 invalid)
            # only row 0 of the 16-channel staging tiles carries data; the
            # scatters read all 16 rows, so initialize them once
            nc.vector.memset(cb16[:], 0.0)
            nc.vector.memset(phi16[:], 0.0)
            nc.vector.memset(plo16[:], 0.0)
            cum_row = cp.tile([1, BL], f32, tag="cum_row")
            dst_row = cp.tile([1, BL], f32, tag="dst_row")
            idx128 = cp.tile([PT, BL], i16, tag="idx128")

            # ---------------- shared pieces ----------------
            def mlp_and_halt(st8, c0, w):
                """MLP head + halt logit for batch cols [c0, c0+w).
                fp8 DoubleRow chains over the fp8 state.
                Writes sigmoid outputs into rows out / h."""
                pm1 = pg.tile([OMID, NBLK], f32, tag="psgi0", name="pm1")
                for kp in range(KP):
                    mlp_mm(pm1, st8, kp, c0, w)
                relu1 = wp.tile([OMID, NBLK], bf16, tag="relu1", name="relu1")
                nc.scalar.activation(relu1[:, :w], pm1[:, :w], AF.Relu,
                                     bias=b1[:, 0:1], scale=SCL)
                ps2 = pg.tile([1, NBLK], f32, tag="psgf0", name="ps2")
                nc.tensor.matmul(ps2[:, :w], W2T[:], relu1[:, :w],
                                 start=True, stop=True)
                psh = pg.tile([1, NBLK], f32, tag="psgc0", name="psh")
                for kp in range(KP):
                    halt_mm(psh, st8, kp, c0, w)
                nc.scalar.activation(row("out", c0, c0 + w), ps2[:, :w], AF.Sigmoid,
                                     bias=sc[0:1, 0:1], scale=1.0)
                nc.scalar.activation(row("h", c0, c0 + w), psh[:, :w], AF.Sigmoid,
                                     bias=sc[0:1, 1:2], scale=SCL)

            def halt_math(c0, w, is_last):
                """Halting update on batch cols [c0, c0+w). Produces contrib row."""
                c1 = c0 + w
                p, a = row("p", c0, c1), row("act", c0, c1)
                omp, fin = row("omp", c0, c1), row("fin", c0, c1)
                hw_, h = row("hw", c0, c1), row("h", c0, c1)
                out, ctr = row("out", c0, c1), row("ctr", c0, c1)
                # omp = 1 - p_sum (old)
                nc.vector.tensor_scalar(out=omp, in0=p, scalar1=-1.0, scalar2=1.0,
                                        op0=ALU.mult, op1=ALU.add)
                # p_sum += h  (now p_new)
                nc.vector.tensor_add(out=p, in0=p, in1=h)
                if is_last:
                    nc.vector.tensor_copy(out=fin, in_=a)
                else:
                    nc.vector.tensor_scalar(out=fin, in0=p, scalar1=1.0 - EPS,
                                            scalar2=None, op0=ALU.is_ge)
                    nc.vector.tensor_mul(out=fin, in0=fin, in1=a)
                # hw = fin ? omp : h  ==  (omp - h)*fin + h   (then mask by act)
                nc.vector.tensor_sub(out=hw_, in0=omp, in1=h)
                nc.vector.tensor_mul(out=hw_, in0=hw_, in1=fin)
                nc.vector.tensor_add(out=hw_, in0=hw_, in1=h)
                nc.vector.tensor_mul(out=hw_, in0=hw_, in1=a)
                nc.vector.tensor_mul(out=ctr, in0=out, in1=hw_)
                nc.vector.tensor_sub(out=a, in0=a, in1=fin)

            def gates_block(t, st, cl, st8, xT, nbs):
                """Gate matmuls + cell/state update for the batch blocks in nbs
                (list of (c0, w)). With >1 block the weight tile loaded for a
                (g, kp, jt) is streamed over every block before moving on,
                amortizing DoubleRow's (FWL-less) weight loads.

                Two phases: (A) per jt: matmul chains, ACT evictions and the
                DVE cell math; (B) tanh(cell) + state mul for all jt."""
                glist = "ico" if t == 0 else GATES
                o_tiles = []
                for jt in range(JT):
                    gt = {g: [] for g in glist}
                    for g in glist:
                        psl = []
                        for bi, (c0, w) in enumerate(nbs):
                            tag = f"psg{g}{bi if len(nbs) > 1 else jt % 2}"
                            psg = pg.tile([PT, NBLK], f32, tag=tag, name=tag)
                            nc.tensor.matmul(psg[:, :w],
                                             WxT[g][:, jt * PT:(jt + 1) * PT],
                                             xT[XOFF[g]:XOFF[g] + IN, c0:c0 + w],
                                             start=True, stop=(t == 0))
                            psl.append(psg)
                        if t > 0:
                            for kp in range(KP):
                                for bi, (c0, w) in enumerate(nbs):
                                    rec_mm(psl[bi], g, kp, jt, st8, c0, w,
                                           stop=(kp == KP - 1))
                        for bi, (c0, w) in enumerate(nbs):
                            gtile = wp.tile([PT, NBLK], bf16, tag=f"g{g}{bi}",
                                            name=f"g{g}{bi}",
                                            bufs=(JT + 1 if g == "o" else None))
                            nc.scalar.activation(gtile[:, :w], psl[bi][:, :w],
                                                 AF.Tanh if g == "c" else AF.Sigmoid,
                                                 bias=bias[g][:, jt:jt + 1],
                                                 scale=SCL)
                            gt[g].append(gtile)
                    for bi, (c0, w) in enumerate(nbs):
                        if t == 0:
                            # cell = i*c
                            nc.vector.tensor_mul(cl[jt][:, c0:c0 + w],
                                                 gt["i"][bi][:, :w],
                                                 gt["c"][bi][:, :w])
                        else:
                            # i *= c ; f *= cell ; cell = i + f  (in gate tiles)
                            nc.vector.tensor_mul(gt["i"][bi][:, :w],
                                                 gt["i"][bi][:, :w],
                                                 gt["c"][bi][:, :w])
                            nc.vector.tensor_mul(gt["f"][bi][:, :w],
                                                 gt["f"][bi][:, :w],
                                                 cl[jt][:, c0:c0 + w])
                            nc.vector.tensor_add(cl[jt][:, c0:c0 + w],
                                                 gt["i"][bi][:, :w],
                                                 gt["f"][bi][:, :w])
                    o_tiles.append(gt["o"])
                for jt in range(JT):
                    for bi, (c0, w) in enumerate(nbs):
                        # state = o * tanh(cell); also refresh the fp8 copy
                        th = wp.tile([PT, NBLK], bf16, tag="th", name="th")
                        nc.scalar.activation(th[:, :w], cl[jt][:, c0:c0 + w],
                                             AF.Tanh)
                        nc.vector.tensor_mul(st[jt][:, c0:c0 + w],
                                             o_tiles[jt][bi][:, :w], th[:, :w])
                        nc.vector.tensor_scalar(out=st8[:, jt, c0:c0 + w],
                                                in0=st[jt][:, c0:c0 + w],
                                                scalar1=SS, scalar2=None,
                                                op0=ALU.mult)

            def acc_add_direct(c0, w):
                c1 = c0 + w
                nc.vector.tensor_add(out=row("acc", c0, c1),
                                     in0=row("acc", c0, c1),
                                     in1=row("ctr", c0, c1))

            # ============ main body (repeatable for slope timing) ============
            REP = [0]

            def main_body():
              nc.vector.memset(row("p"), 0.0)
              nc.vector.memset(row("act"), 1.0)
              nc.vector.memset(row("acc"), 0.0)
              def dense_iter(t):
                  # all gate matmuls first so the PE stream never waits on the
                  # DVE state-update epilogue of the previous batch block
                  if opts["pair"]:
                      gates_block(t, stA, clA, st8A, xT_A,
                                  [(0, NBLK), (NBLK, NBLK)])
                      for nb in range(NBLKS):
                          mlp_and_halt(st8A, nb * NBLK, NBLK)
                          halt_math(nb * NBLK, NBLK, is_last=False)
                          acc_add_direct(nb * NBLK, NBLK)
                  elif opts["gates_first"]:
                      for nb in range(NBLKS):
                          gates_block(t, stA, clA, st8A, xT_A,
                                      [(nb * NBLK, NBLK)])
                      for nb in range(NBLKS):
                          mlp_and_halt(st8A, nb * NBLK, NBLK)
                          halt_math(nb * NBLK, NBLK, is_last=False)
                          acc_add_direct(nb * NBLK, NBLK)
                  else:
                      for nb in range(NBLKS):
                          gates_block(t, stA, clA, st8A, xT_A,
                                      [(nb * NBLK, NBLK)])
                          mlp_and_halt(st8A, nb * NBLK, NBLK)
                          halt_math(nb * NBLK, NBLK, is_last=False)
                          acc_add_direct(nb * NBLK, NBLK)

              # ------------- t = 0 (state==0: x-projection only) -------------
              dense_iter(0)

              # ---------------- t = 1 (dense) ----------------
              if upto == 't0':
                  return
              dense_iter(1)

              if upto == 't1':
                  return

              # total active count -> one register on every engine (fresh
              # tiles per call: the raw reg_load reads are not fully
              # WAR-tracked by Tile)
              def count_total(tagix):
                  cntf = cp.tile([1, 8], f32, tag=f"cntf{REP[0]}_{tagix}", name=f"cntf{REP[0]}_{tagix}")
                  cnti = cp.tile([1, 8], i32, tag=f"cnti{REP[0]}_{tagix}", name=f"cnti{REP[0]}_{tagix}")
                  nc.vector.reduce_sum(out=cntf[0:1, 0:1], in_=row("act"),
                                       axis=mybir.AxisListType.X)
                  nc.vector.tensor_copy(out=cnti[0:1, 0:1], in_=cntf[0:1, 0:1])
                  return nc.values_load(cnti[0:1, 0:1], min_val=0, max_val=BL,
                                        skip_runtime_bounds_check=True)

              def compaction_idx_half(nb):
                  # prefix-sum of act for one 512-col half (chained via the
                  # previous half's last element); dest = cumsum*act - 1.
                  # Half 0 only depends on t1 block 0's halting update, so it
                  # hides behind t1 block 1's matmuls.
                  c0, c1 = nb * NBLK, (nb + 1) * NBLK
                  nc.vector.tensor_tensor_scan(
                      out=cum_row[0:1, c0:c1], data0=row("act", c0, c1),
                      data1=row("act", c0, c1),
                      initial=0.0 if nb == 0 else cum_row[0:1, c0 - 1:c0],
                      op0=ALU.add, op1=ALU.max)
                  dst = dst_row[0:1, c0:c1]
                  nc.vector.tensor_mul(out=dst, in0=cum_row[0:1, c0:c1],
                                       in1=row("act", c0, c1))
                  nc.vector.tensor_scalar_add(out=dst, in0=dst, scalar1=-1.0)
                  # broadcast dest to 128 partitions (ones outer product; f32
                  # matmul is exact for these small integers)
                  ptag = "psgi0" if nb == 0 else "psgf0"
                  pb = pg.tile([PT, NBLK], f32, tag=ptag, name=f"pb{nb}")
                  nc.tensor.matmul(pb[:], ones128[:], dst, start=True, stop=True)
                  nc.vector.tensor_copy(out=idx128[:, c0:c1], in_=pb[:])

              def compaction():
                  # compact x first (t2's chains start from it), then state in
                  # chain order (dst k is src k+1), then cell; after each state
                  # tile lands, refresh the fp8 DoubleRow copy (full width so
                  # guarded t>=3 blocks never read uninitialized fp8)
                  nc.gpsimd.local_scatter(xT_B[:], xT_A[:], idx128[:],
                                          2 * IN, BL, BL)
                  for kt in range(KT):
                      nc.gpsimd.local_scatter(stB[kt][:], stA[kt][:], idx128[:],
                                              PT, BL, BL)
                      nc.vector.tensor_scalar(out=st8B[:, kt, :],
                                              in0=stB[kt][:],
                                              scalar1=SS, scalar2=None,
                                              op0=ALU.mult)
                  for kt in range(KT):
                      nc.gpsimd.local_scatter(clB[kt][:], clA[kt][:], idx128[:],
                                              PT, BL, BL)
                  # compact p_sum as bf16 hi+lo split (exact to ~2^-17)
                  nc.vector.tensor_copy(out=phi16[0:1, :], in_=row("p"))
                  nc.vector.tensor_sub(out=plo16[0:1, :], in0=row("p"),
                                       in1=phi16[0:1, :])
                  nc.gpsimd.local_scatter(sct16[:], phi16[:], idx128[0:16, :],
                                          16, BL, BL)
                  nc.vector.tensor_copy(out=phi16[0:1, :], in_=sct16[0:1, :])
                  nc.gpsimd.local_scatter(sct16[:], plo16[:], idx128[0:16, :],
                                          16, BL, BL)
                  nc.vector.tensor_add(out=row("p"), in0=phi16[0:1, :],
                                       in1=sct16[0:1, :])
                  # compact original row ids (1-based -> 0-based; empty -> -1)
                  nc.gpsimd.local_scatter(orig16[:], iota1[:], idx128[0:16, :],
                                          16, BL, BL)
                  nc.vector.tensor_scalar(out=orig16[:], in0=orig16[:], scalar1=1,
                                          scalar2=None, op0=ALU.subtract)
                  # act := (slot occupied) in compact space
                  nc.vector.tensor_scalar(out=row("act"), in0=orig16[0:1, :],
                                          scalar1=0, scalar2=None, op0=ALU.is_ge)

              if upto == 'compact':
                  compaction_idx_half(0)
                  compaction_idx_half(1)
                  compaction()
                  return

              def block_work(t, c0, w):
                  gates_block(t, stB, clB, st8B, xT_B, [(c0, w)])
                  mlp_and_halt(st8B, c0, w)
                  halt_math(c0, w, is_last=(t == MAX_ITER - 1))
                  nc.vector.tensor_copy(out=cb16[0:1, c0:c0 + w],
                                        in_=row("ctr", c0, c0 + w))

              def acc_scatter():
                  # scatter contributions back to original row order
                  nc.gpsimd.local_scatter(sct16[:], cb16[:], orig16[:],
                                          16, BL, BL)
                  nc.vector.tensor_add(out=row("acc"), in0=row("acc"),
                                       in1=sct16[0:1, :])

              # timing variant: guard-free tail (correct only when every row
              # halts by t=2 in <=384 compact columns -- measurement only)
              if upto == 'noif':
                  compaction_idx_half(0)
                  compaction_idx_half(1)
                  compaction()
                  nc.vector.memset(cb16[0:1, :], 0.0)
                  block_work(2, 0, 384)
                  acc_scatter()
                  return

              # ---------------- t = 2..7 (compact space, fully guarded) -------
              def late_iter(t, cnt):
                  pf_skip = False if opts["ft_hints"] else None
                  with tc.If(cnt > 0, preferred_fallthrough_block=pf_skip):
                      nc.vector.memset(cb16[0:1, :], 0.0)
                      # actives may sit anywhere in compact space at t>=3 (no
                      # recompaction); run both halves under the one guard
                      block_work(t, 0, NBLK)
                      block_work(t, NBLK, NBLK)
                      acc_scatter()
                      if t < MAX_ITER - 1:
                          late_iter(t + 1, count_total(t))

              # t=2: compaction runs inside the guard (skipped if all rows
              # halted) so the scatters pipeline with t2's first matmul chains;
              # in compact space actives are contiguous, so block0 needs no
              # guard and later blocks guard on n2 thresholds.
              compaction_idx_half(0)
              compaction_idx_half(1)
              n2 = count_total(1)
              pf2 = True if opts["ft_hints"] else None
              pf_skip2 = False if opts["ft_hints"] else None
              # t2 block plan: compact actives are contiguous from col 0, so
              # use finer 384/384/256 blocks, each guarded by n2 thresholds
              with tc.If(n2 > 0, preferred_fallthrough_block=pf2):
                  compaction()
                  nc.vector.memset(cb16[0:1, :], 0.0)
                  block_work(2, 0, 384)
                  with tc.If(n2 > 384, preferred_fallthrough_block=pf_skip2):
                      block_work(2, 384, 384)
                  with tc.If(n2 > 768, preferred_fallthrough_block=pf_skip2):
                      block_work(2, 768, 256)
                  acc_scatter()
                  late_iter(3, count_total(2))

            for _rep in range(reps):
                REP[0] = _rep
                main_body()

            # ---------------- output ----------------
            nc.sync.dma_start(out=d_out.ap()[:, :], in_=row("acc"))

    nc.compile()
    return nc


def _interleave_dr(WT, m):
    """WT: [HID, m] k-major -> [128, KP, 2m] fp8, SwInterleave layout:
    [p, kp, 2j] = WT[2kp*128+p, m-1-j]*WS, [p, kp, 2j+1] = WT[(2kp+1)*128+p, m-1-j]."""
    f8 = ml_dtypes.float8_e4m3
    a = (WT.astype(np.float32) * WS).reshape(KP, 2, PT, m)  # [kp, i, p, j]
    out = np.zeros((PT, KP, 2 * m), np.float32)
    out[:, :, 0::2] = a[:, 0].transpose(1, 0, 2)[:, :, ::-1]
    out[:, :, 1::2] = a[:, 1].transpose(1, 0, 2)[:, :, ::-1]
    return np.clip(out, -240.0, 240.0).astype(f8)


def _prep_inputs(x, Wxi, bxi, Whi, bhi, Wxf, bxf, Whf, bhf, Wxc, bxc, Whc, bhc,
                 Wxo, bxo, Who, bho, W_halt, b_halt, W1, b1, W2, b2):
    bf = ml_dtypes.bfloat16
    f8 = ml_dtypes.float8_e4m3
    XS = np.float32(WS * SS)  # x-projection weight pre-scale (2^19)

    def to_dr(WT, m):
        # WT: [HID, m] (k-major) -> [128, KT, m] fp8 with [p, k, j] = WT[k*128+p, j]*WS
        a = (WT.astype(np.float32) * WS).reshape(KT, PT, m).transpose(1, 0, 2)
        return np.clip(a, -240.0, 240.0).astype(f8)

    gw = {"i": (Wxi, bxi, Whi, bhi), "f": (Wxf, bxf, Whf, bhf),
          "c": (Wxc, bxc, Whc, bhc), "o": (Wxo, bxo, Who, bho)}
    shared = {}
    for g, (Wx, bx, Wh, bh) in gw.items():
        WhT = Wh.T
        shared[f"Wh8{g}"] = np.ascontiguousarray(to_dr(WhT, HID))
        # SwInterleave layout is per (kp, jt) block of 128 output cols
        swi = np.zeros((PT, KP * JT, 2 * PT), ml_dtypes.float8_e4m3)
        for jt in range(JT):
            blk = _interleave_dr(WhT[:, jt * PT:(jt + 1) * PT], PT)  # [128,KP,256]
            for kp in range(KP):
                swi[:, kp * JT + jt, :] = blk[:, kp, :]
        shared[f"Wh8s{g}"] = np.ascontiguousarray(swi)
        shared[f"bias{g}"] = np.ascontiguousarray(
            (bx + bh).astype(np.float32).reshape(JT, PT).T)
    shared["WxTif"] = np.ascontiguousarray(
        np.concatenate([gw["i"][0].T, gw["f"][0].T], axis=0)
        .astype(np.float32) * XS).astype(bf)
    shared["WxTco"] = np.ascontiguousarray(
        np.concatenate([gw["c"][0].T, gw["o"][0].T], axis=0)
        .astype(np.float32) * XS).astype(bf)
    shared["W18"] = np.ascontiguousarray(to_dr(W1.T, OMID))
    shared["W18s"] = np.ascontiguousarray(_interleave_dr(W1.T, OMID))
    shared["b1"] = b1.astype(np.float32).reshape(OMID, 1)
    shared["W2T"] = np.ascontiguousarray(W2.T).astype(bf)
    whalt = np.zeros((HID, 16), np.float32)
    whalt[:, 0] = W_halt.astype(np.float32).reshape(HID)
    shared["Whalt8"] = np.ascontiguousarray(to_dr(whalt, 16))
    # halt SwInterleave: m=1 -> per kp just [A_0, B_0] in cols 0:2 (16-wide pad)
    whs = np.zeros((PT, KP, 16), np.float32)
    wh = (W_halt.astype(np.float32).reshape(KP, 2, PT) * WS)
    whs[:, :, 0] = wh[:, 0].T
    whs[:, :, 1] = wh[:, 1].T
    shared["Whalt8s"] = np.ascontiguousarray(whs.astype(f8))
    shared["sc"] = np.array([[b2[0], b_halt[0]]], dtype=np.float32)
    shared["iota1"] = np.tile(np.arange(1, BL + 1, dtype=np.int16), (16, 1))
    shared["ones128"] = np.ones((1, PT), dtype=np.float32)

    in_maps = []
    for c in range(NCORES):
        m = dict(shared)
        xs = x[c * BL:(c + 1) * BL].astype(np.float32)
        xt = np.ascontiguousarray(xs.T).astype(bf)
        m["xT"] = np.concatenate([xt, xt], axis=0)
        in_maps.append(m)
    return in_maps


def kernel(**inputs):
    from concourse.bass_utils import run_bass_kernel_spmd

    if "nc" not in _CACHE:
        _CACHE["nc"] = _build_nc()
    nc = _CACHE["nc"]

    in_maps = _prep_inputs(**{k: np.asarray(v) for k, v in inputs.items()})
    res = run_bass_kernel_spmd(nc, in_maps, core_ids=list(range(NCORES)))
    out = np.concatenate([res.results[c]["out"][0] for c in range(NCORES)])
    return out.reshape(BATCH, 1).astype(np.float32)
